# revision 1
# baseline (speedup 1.0000x reference)
"""Trainium2 Bass kernel for KernelAttention (gaussian-kernel multi-head attention).

Math (per batch b):
  d2[q,k]   = |q_pos[q] - k_pos[k]|^2   (computed as -d2 via one K=5 augmented matmul)
  s_h[k,q]  = exp(-c_h * d2),  c_h = 1/lengthscale_h^2   (masked keys contribute 0)
  att_h[q,v]= sum_k s_h[k,q] * V[k,h,v] / (sum_k s_h[k,q]*unmasked[k] + 1e-5)
  out[q,o]  = sum_{h,v} att_h[q,v] * w_out[o, h*64+v]

Sharding: 8 cores = (batch b in 0..3) x (query half in 0..1); each core owns
[1024 q, 2048 k]. All inputs host-prepped per core; outputs gathered on host.

Device-side layout is score-transposed: s_h is [k, q] so the attend matmul
(lhsT = values, rhs = scores) contracts k on the PE partition dim directly.
Masking + normalization are folded in: masked V rows are zeroed on the host and
a per-head ones-column (also mask-zeroed) produces the normalizer as psum row 64.
Normalization is deferred past the attend; the reciprocal is computed via
ACT Ln/Exp and broadcast across partitions with a tiny K=8 selection matmul.
Two heads (c=4, c=0.04) are derived from their 4x-smaller neighbors by two DVE
squarings, offloading exp work from the scalar engine.
"""

import numpy as np
from contextlib import ExitStack

B, LQ, LK, DPOS = 4, 2048, 2048, 3
H, V, OUTD = 8, 64, 512
QS = LQ // 2          # q rows per core
KT = LK // 128        # k tiles
V1 = V + 1            # value cols + ones col
NCORES = 8

# head processing order: chain sources immediately before their derived heads
ORDER = [3, 2, 6, 5, 0, 1, 4, 7]
DERIVED = {2: 3, 5: 6}  # derived_head -> source_head, s_d = s_src ** 4

_cache = {}


def _build(cv, use_chains):
    key = (tuple(cv), use_chains)
    if key in _cache:
        return _cache[key]
    import concourse.bacc as bacc
    import concourse.tile as tile
    from concourse import mybir

    f32 = mybir.dt.float32
    bf16 = mybir.dt.bfloat16
    AF = mybir.ActivationFunctionType

    nc = bacc.Bacc("TRN2", target_bir_lowering=False, debug=False,
                   num_devices=NCORES)
    # ka/qa carry a hi/lo bf16 split of the K=5 augmented distance operands:
    # rows [hi(5); lo(5); hi(5)] x [hi(5); hi(5); lo(5)] so the single bf16
    # matmul accumulates hi*hi + lo*hi + hi*lo in fp32 PSUM (lo*lo ~2^-16 is
    # dropped). This avoids fp32 LOW_HIGH double-pass matmuls entirely.
    ka = nc.dram_tensor("ka", [15, LK], bf16, kind="ExternalInput").ap()
    qa = nc.dram_tensor("qa", [15, QS], bf16, kind="ExternalInput").ap()
    vp = nc.dram_tensor("vp", [128, KT, H * V1], bf16, kind="ExternalInput").ap()
    wt = nc.dram_tensor("wt", [128, 4, OUTD], bf16, kind="ExternalInput").ap()
    sel8 = nc.dram_tensor("sel8", [8, 4, 128], bf16, kind="ExternalInput").ap()
    outT = nc.dram_tensor("outT", [OUTD, QS], f32, kind="ExternalOutput").ap()

    with tile.TileContext(nc) as tc, ExitStack() as ctx:
        const = ctx.enter_context(tc.tile_pool(name="const", bufs=1))
        spool = ctx.enter_context(tc.tile_pool(name="spool", bufs=10))
        stage = ctx.enter_context(tc.tile_pool(name="stage", bufs=2))
        obuf = ctx.enter_context(tc.tile_pool(name="obuf", bufs=2))
        psp = ctx.enter_context(tc.tile_pool(name="psum", bufs=4, space="PSUM"))

        ka_sb = const.tile([15, LK], bf16)
        nc.sync.dma_start(out=ka_sb[:], in_=ka)
        qa_sb = const.tile([15, QS], bf16)
        nc.sync.dma_start(out=qa_sb[:], in_=qa)
        vp_sb = const.tile([128, KT, H * V1], bf16)
        nc.sync.dma_start(out=vp_sb[:], in_=vp)
        wt_sb = const.tile([128, 4, OUTD], bf16)
        nc.sync.dma_start(out=wt_sb[:], in_=wt)
        sel8_sb = const.tile([8, 4, 128], bf16)
        nc.sync.dma_start(out=sel8_sb[:], in_=sel8)

        # Phase A: m = -d2 in [k, q] layout, evacuated to bf16 SBUF.
        # m is split into NG group tiles (4 k-tiles each) so per-head exp /
        # squaring / attend pipeline at ~3.7us granularity — PE never idles
        # longer than the HAM MID window, staying at full clock.
        NG, GK = 4, KT // 4
        m_g = [const.tile([128, GK, QS], bf16, tag=f"m{g}", name=f"m{g}")
               for g in range(NG)]
        for kt in range(KT):
            d2 = psp.tile([128, QS], f32, tag="ps")
            for qc in range(2):
                s5 = slice(qc * 512, (qc + 1) * 512)
                nc.tensor.matmul(d2[:, s5],
                                 lhsT=ka_sb[:, kt * 128:(kt + 1) * 128],
                                 rhs=qa_sb[:, s5], start=True, stop=True)
            nc.vector.tensor_copy(out=m_g[kt // GK][:, kt % GK, :], in_=d2[:])

        flat = [const.tile([128, QS], bf16, tag=f"flat{j}", name=f"flat{j}")
                for j in range(4)]
        norms = const.tile([8, QS], f32)
        nc.vector.memset(norms[:], 1.0)
        eps_t = const.tile([8, 1], f32)
        nc.vector.memset(eps_t[:], 1e-5)
        lnn = const.tile([8, QS], f32)
        r_all = const.tile([8, QS], f32)
        r_hi = const.tile([8, QS], bf16)
        nc.vector.memset(r_hi[:], 0.0)
        r_lo = const.tile([8, QS], bf16)
        nc.vector.memset(r_lo[:], 0.0)

        s_tiles = {}
        for h in ORDER:
            sg = []
            att = psp.tile([V1, QS], f32, tag="ps")
            for g in range(NG):
                s = spool.tile([128, GK, QS], bf16, tag="s", name=f"s{h}_{g}")
                if use_chains and h in DERIVED:
                    src = s_tiles[DERIVED[h]][g]
                    nc.vector.tensor_mul(s[:], src[:], src[:])
                    nc.vector.tensor_mul(s[:], s[:], s[:])
                else:
                    nc.scalar.activation(out=s[:], in_=m_g[g][:], func=AF.Exp,
                                         scale=float(cv[h]))
                sg.append(s)
                for qc in range(2):
                    s5 = slice(qc * 512, (qc + 1) * 512)
                    for k2 in range(GK):
                        kt = g * GK + k2
                        nc.tensor.matmul(att[:, s5],
                                         lhsT=vp_sb[:, kt, h * V1:(h + 1) * V1],
                                         rhs=s[:, k2, s5],
                                         start=(kt == 0), stop=(kt == KT - 1))
            s_tiles[h] = sg
            r0 = (h % 2) * 64
            nc.vector.tensor_copy(out=flat[h // 2][r0:r0 + 64, :],
                                  in_=att[0:64, :])
            stg = stage.tile([V1, QS], f32, tag="stg")
            nc.vector.tensor_copy(out=stg[64:65, :], in_=att[64:65, :])
            nc.sync.dma_start(out=norms[h:h + 1, :], in_=stg[64:65, :])

        # r = 1/(norm + 1e-5) via exp(-ln(x)); Ln+Exp share one ACT table set
        nc.scalar.activation(out=lnn[:], in_=norms[:], func=AF.Ln, bias=eps_t[:])
        nc.scalar.activation(out=r_all[:], in_=lnn[:], func=AF.Exp, scale=-1.0)
        nc.vector.tensor_copy(out=r_hi[:], in_=r_all[:])
        nc.vector.tensor_sub(r_lo[:], r_all[:], r_hi[:])
        # broadcast r across partitions (head pair j -> 128 rows) and normalize
        for j in range(4):
            rb = psp.tile([128, QS], f32, tag="ps", name=f"rb{j}")
            for qc in range(2):
                s5 = slice(qc * 512, (qc + 1) * 512)
                nc.tensor.matmul(rb[:, s5], lhsT=sel8_sb[:, j, :],
                                 rhs=r_hi[:, s5], start=True, stop=False)
                nc.tensor.matmul(rb[:, s5], lhsT=sel8_sb[:, j, :],
                                 rhs=r_lo[:, s5], start=False, stop=True)
            nc.vector.tensor_mul(flat[j][:], flat[j][:], rb[:])

        # out projection: outT[o, q] = sum_hv wt[hv, o] * flat[hv, q]
        for ot in range(4):
            po = psp.tile([128, QS], f32, tag="ps", name=f"po{ot}")
            for j in range(4):
                for qc in range(2):
                    s5 = slice(qc * 512, (qc + 1) * 512)
                    nc.tensor.matmul(po[:, s5],
                                     lhsT=wt_sb[:, j, ot * 128:(ot + 1) * 128],
                                     rhs=flat[j][:, s5],
                                     start=(j == 0), stop=(j == 3))
            ob = obuf.tile([128, QS], f32, tag="ob", name=f"ob{ot}")
            if ot % 2 == 0:
                nc.scalar.copy(out=ob[:], in_=po[:])
            else:
                nc.vector.tensor_copy(out=ob[:], in_=po[:])
            nc.sync.dma_start(out=outT[ot * 128:(ot + 1) * 128, :], in_=ob[:])

    nc.compile()
    _cache[key] = nc
    return nc


def _hilo(x, bf16):
    hi = x.astype(bf16)
    lo = (x - hi.astype(np.float32)).astype(bf16)
    return hi, lo


def _prep_core(qp, kp, vals, mask, w_out, bf16):
    q2 = (qp * qp).sum(-1)
    one_q = np.ones(QS, np.float32)
    qa5 = np.stack([2 * qp[:, 0], 2 * qp[:, 1], 2 * qp[:, 2], -one_q, -q2]) \
        .astype(np.float32)
    k2 = (kp * kp).sum(-1)
    one_k = np.ones(LK, np.float32)
    ka5 = np.stack([kp[:, 0], kp[:, 1], kp[:, 2], k2, one_k]).astype(np.float32)
    ka_hi, ka_lo = _hilo(ka5, bf16)
    qa_hi, qa_lo = _hilo(qa5, bf16)
    ka = np.concatenate([ka_hi, ka_lo, ka_hi])   # [15, LK]
    qa = np.concatenate([qa_hi, qa_hi, qa_lo])   # [15, QS]
    vv = np.concatenate([vals, np.ones((LK, H, 1), np.float32)], axis=-1)
    vv = vv.copy()
    vv[mask] = 0.0
    vp = vv.reshape(KT, 128, H * V1).transpose(1, 0, 2).astype(bf16)
    wt = np.ascontiguousarray(w_out.T).reshape(4, 128, OUTD) \
        .transpose(1, 0, 2).astype(bf16)
    sel8 = np.zeros((8, 4, 128), np.float32)
    for j in range(4):
        sel8[2 * j, j, :64] = 1.0
        sel8[2 * j + 1, j, 64:] = 1.0
    return {"ka": np.ascontiguousarray(ka), "qa": np.ascontiguousarray(qa),
            "vp": np.ascontiguousarray(vp), "wt": np.ascontiguousarray(wt),
            "sel8": sel8.astype(bf16)}


def kernel(query_positions, key_positions, values, masked_elements,
           lengthscales, w_out, _want_trace=False):
    import ml_dtypes
    from concourse.bass_utils import run_bass_kernel_spmd

    bf16 = ml_dtypes.bfloat16
    qp = np.asarray(query_positions, np.float32)
    kp = np.asarray(key_positions, np.float32)
    vals = np.asarray(values, np.float32)
    mask = np.asarray(masked_elements).astype(bool)
    ls = np.asarray(lengthscales, np.float32)
    w = np.asarray(w_out, np.float32)

    cv = (1.0 / (ls.astype(np.float64) ** 2)).astype(np.float32)
    use_chains = all(
        np.float32(cv[d]) == np.float32(4.0) * np.float32(cv[s])
        for d, s in DERIVED.items())
    nc = _build(tuple(float(x) for x in cv), use_chains)

    in_maps = []
    for c in range(NCORES):
        b, hf = c // 2, c % 2
        in_maps.append(_prep_core(qp[b, hf * QS:(hf + 1) * QS], kp[b],
                                  vals[b], mask[b], w, bf16))
    res = run_bass_kernel_spmd(nc, in_maps, core_ids=list(range(NCORES)),
                               trace=_want_trace)
    out = np.empty((B, LQ, OUTD), np.float32)
    for c in range(NCORES):
        b, hf = c // 2, c % 2
        out[b, hf * QS:(hf + 1) * QS, :] = res.results[c]["outT"].T
    if _want_trace:
        return out, res
    return out



# revision 2
# speedup vs baseline: 1.0311x; 1.0311x over previous
"""Trainium2 Bass kernel for KernelAttention — v12.

v3 vs v2:
- dist/exp pipeline: 2-kt pairs with alternating PSUM tags (dA/dB) so the
  next pair's distance matmul runs while ACT consumes the previous pair
  (ACT back-to-back instead of 59% busy).
- even-head norm rows ride the flat evacuation ([0:65] copy) and are DMA'd
  out of SBUF before the odd head overwrites row 64; only odd heads pay a
  [1,512] staging copy.
- chain-head attends interleaved into the pass1 pair loop (PE stays warm).
- GPSIMD absorbs the t2 squarings of head0-q0 (slack path).
"""

import numpy as np
from contextlib import ExitStack

B, LQ, LK, DPOS = 4, 2048, 2048, 3
H, V, OUTD = 8, 64, 512
QS = LQ // 2
KT = LK // 128
V1 = V + 1
NCORES = 8
NP, PK = 8, 2            # 8 pairs of 2 k-tiles per q-half
NG, GK = 4, 4            # chain/att grouping stays 4-kt

EXP_HEADS = [3, 1, 4]
CHAIN = [(2, 3), (0, 1)]
POLY_DEG = {5: 5, 6: 3, 7: 2}
POLY_BASE = {5: 0, 6: 64, 7: 96}

_cache = {}


def _mono_exps(deg):
    out = []
    for n in range(deg + 1):
        for i in range(n + 1):
            for j in range(n - i + 1):
                out.append((i, j, n - i - j))
    return out


def _build(cv):
    key = tuple(cv)
    if key in _cache:
        return _cache[key]
    import concourse.bacc as bacc
    import concourse.tile as tile
    from concourse import mybir

    f32 = mybir.dt.float32
    bf16 = mybir.dt.bfloat16
    AF = mybir.ActivationFunctionType

    nc = bacc.Bacc("TRN2", target_bir_lowering=False, debug=False,
                   num_devices=NCORES)
    ka4 = nc.dram_tensor("ka4", [64, NP, 128], bf16, kind="ExternalInput").ap()
    qa4 = nc.dram_tensor("qa4", [64, QS], bf16, kind="ExternalInput").ap()
    vp = nc.dram_tensor("vp", [128, KT, 5 * V1], bf16, kind="ExternalInput").ap()
    pv = nc.dram_tensor("pv", [128, KT, 3 * V1], bf16, kind="ExternalInput").ap()
    phi = nc.dram_tensor("phi", [128, KT, 128], bf16, kind="ExternalInput").ap()
    psi = nc.dram_tensor("psi", [128, QS], bf16, kind="ExternalInput").ap()
    wt = nc.dram_tensor("wt", [128, 4, OUTD], bf16, kind="ExternalInput").ap()
    sel8 = nc.dram_tensor("sel8", [8, 4, 128], bf16, kind="ExternalInput").ap()
    outT = nc.dram_tensor("outT", [OUTD, QS], f32, kind="ExternalOutput").ap()

    DIDX = {0: 0, 1: 1, 2: 2, 3: 3, 4: 4}
    PIDX = {5: 0, 6: 1, 7: 2}
    PCNT = {h: len(_mono_exps(d)) for h, d in POLY_DEG.items()}

    with tile.TileContext(nc) as tc, ExitStack() as ctx:
        const = ctx.enter_context(tc.tile_pool(name="const", bufs=1))
        spool = ctx.enter_context(tc.tile_pool(name="spool", bufs=2))
        tpool = ctx.enter_context(tc.tile_pool(name="tpool", bufs=2))
        obuf = ctx.enter_context(tc.tile_pool(name="obuf", bufs=4))
        psA = ctx.enter_context(tc.tile_pool(name="psA", bufs=1, space="PSUM"))
        psB = ctx.enter_context(tc.tile_pool(name="psB", bufs=1, space="PSUM"))
        psX = ctx.enter_context(tc.tile_pool(name="psX", bufs=1, space="PSUM"))

        ka_sb = const.tile([64, NP, 128], bf16)
        nc.sync.dma_start(out=ka_sb[:], in_=ka4)
        qa_sb = const.tile([64, QS], bf16)
        nc.sync.dma_start(out=qa_sb[:], in_=qa4)
        vp_sb = const.tile([128, KT, 5 * V1], bf16)
        nc.sync.dma_start(out=vp_sb[:, 0:4, :], in_=vp[:, 0:4, :])
        nc.sync.dma_start(out=vp_sb[:, 4:16, :], in_=vp[:, 4:16, :])
        phi_sb = const.tile([128, KT, 128], bf16)
        nc.sync.dma_start(out=phi_sb[:], in_=phi)
        pv_sb = const.tile([128, KT, 3 * V1], bf16)
        nc.sync.dma_start(out=pv_sb[:], in_=pv)
        psi_sb = const.tile([128, QS], bf16)
        nc.sync.dma_start(out=psi_sb[:], in_=psi)
        wt_sb = const.tile([128, 4, OUTD], bf16)
        nc.sync.dma_start(out=wt_sb[:], in_=wt)
        sel8_sb = const.tile([8, 4, 128], bf16)
        nc.sync.dma_start(out=sel8_sb[:], in_=sel8)

        flat = [const.tile([128, QS], bf16, name=f"flat{j}") for j in range(4)]
        norms = const.tile([8, QS], f32)
        nc.gpsimd.memset(norms[:], 1.0)
        nrm_e = const.tile([8, QS], f32)
        r_all = const.tile([8, QS], f32)
        r_hi = const.tile([8, QS], bf16)
        r_lo = const.tile([8, QS], bf16)
        m_sb = const.tile([128, 3 * V1], bf16)
        stg = const.tile([V1, 8, 512], f32)

        s_store, att_t = {}, {}

        def qsl(qc):
            return slice(qc * 512, (qc + 1) * 512)

        def dist_p(qc, p):
            d2p = psA.tile([128, PK, 512], f32, tag=f"d{p % 2}",
                           name=f"d2_q{qc}p{p}")
            for i in range(PK):
                bp = 32 * i
                nc.tensor.matmul(d2p[:, i, :], lhsT=ka_sb[bp:bp + 15, p, :],
                                 rhs=qa_sb[bp:bp + 15, qsl(qc)],
                                 start=True, stop=True, tile_position=(bp, 0))
            return d2p

        def exps_p(qc, p, d2p):
            for h in EXP_HEADS:
                nc.scalar.activation(
                    out=s_store[(h, qc)][:, p * PK:(p + 1) * PK, :],
                    in_=d2p[:, :, :], func=AF.Exp, scale=float(cv[h]))

        def att_p(att, h, s_tile, p):
            for k2 in range(PK):
                kt = p * PK + k2
                nc.tensor.matmul(att[0:V1, :],
                                 lhsT=vp_sb[:, kt, DIDX[h] * V1:(DIDX[h] + 1) * V1],
                                 rhs=s_tile[:, kt, :],
                                 start=(kt == 0), stop=(kt == KT - 1))

        def att_chain_g(att, dst, sd, g):
            for k2 in range(GK):
                kt = g * GK + k2
                nc.tensor.matmul(att[0:V1, :],
                                 lhsT=vp_sb[:, kt, DIDX[dst] * V1:(DIDX[dst] + 1) * V1],
                                 rhs=sd[:, k2, :],
                                 start=(kt == 0), stop=(kt == KT - 1))

        def evac_lo(att, h, qc, eng):
            j = h // 2
            cs = qsl(qc)
            cp = nc.vector.tensor_copy if eng == "v" else nc.scalar.copy
            cp(out=flat[j][0:V1, cs], in_=att[0:V1, :])
            nc.gpsimd.dma_start(out=norms[h:h + 1, cs], in_=flat[j][64:65, cs])

        def evac_hi(att, h, qc, eng):
            j = h // 2
            cs = qsl(qc)
            cp = nc.vector.tensor_copy if eng == "v" else nc.scalar.copy
            fl65 = tpool.tile([V1, 512], f32, tag="fl65", name=f"fl65_{h}q{qc}")
            cp(out=fl65[:], in_=att[0:V1, :])
            nc.sync.dma_start(out=norms[h:h + 1, cs], in_=fl65[64:65, :])
            nc.gpsimd.dma_start(out=flat[j][64:128, cs], in_=fl65[0:64, :])

        def chain_g(dst, src, qc, g, t2_eng="v"):
            gs = slice(g * GK, (g + 1) * GK)
            t2 = tpool.tile([128, GK, 512], bf16, tag="t2",
                            name=f"t2_{dst}q{qc}g{g}")
            sd = tpool.tile([128, GK, 512], bf16, tag=f"sd{dst}",
                            name=f"sd{dst}_q{qc}g{g}")
            s_src = s_store[(src, qc)]
            if t2_eng == "g":
                nc.gpsimd.tensor_mul(t2[:], s_src[:, gs, :], s_src[:, gs, :])
            else:
                nc.vector.tensor_mul(t2[:], s_src[:, gs, :], s_src[:, gs, :])
            nc.vector.tensor_mul(sd[:], t2[:], t2[:])
            return sd

        def patt_emit(qc, tags):
            tiles = []
            for h in POLY_DEG:
                att = psB.tile([V1, 512], f32, tag=tags[PIDX[h]],
                               name=f"patt{h}_q{qc}")
                b0, pn, pi = POLY_BASE[h], PCNT[h], PIDX[h]
                nc.tensor.matmul(att[0:V1, :],
                                 lhsT=m_sb[b0:b0 + pn, pi * V1:(pi + 1) * V1],
                                 rhs=psi_sb[b0:b0 + pn, qsl(qc)],
                                 start=True, stop=True, tile_position=(b0, 0))
                tiles.append((att, h))
            return tiles

        def recip_q(qc):
            cs = qsl(qc)
            nc.vector.tensor_scalar_add(nrm_e[:, cs], norms[:, cs], 1e-5)
            nc.vector.reciprocal_approx_fast(out=r_all[:, cs], in_=nrm_e[:, cs])
            nc.vector.tensor_copy(out=r_hi[:, cs], in_=r_all[:, cs])

        def rb_j(qc, j, pool_tile):
            cs = qsl(qc)
            nc.tensor.matmul(pool_tile[:], lhsT=sel8_sb[:, j, :],
                             rhs=r_hi[:, cs], start=True, stop=True)

        def proj_ot(qc, ot, pool_tile, eng):
            cs = qsl(qc)
            for j in range(4):
                nc.tensor.matmul(pool_tile[:],
                                 lhsT=wt_sb[:, j, ot * 128:(ot + 1) * 128],
                                 rhs=flat[j][:, cs], start=(j == 0), stop=(j == 3))
            ob = obuf.tile([128, 512], f32, tag="ob", name=f"ob{ot}_q{qc}")
            if eng == "v":
                nc.vector.tensor_copy(out=ob[:], in_=pool_tile[:])
            else:
                nc.scalar.copy(out=ob[:], in_=pool_tile[:])
            nc.sync.dma_start(out=outT[ot * 128:(ot + 1) * 128, cs], in_=ob[:])

        # ---------------- emission ----------------
        for qc in range(2):
            for h in set(EXP_HEADS):
                s_store[(h, qc)] = spool.tile([128, KT, 512], bf16,
                                              tag=f"s{h}", name=f"s{h}_q{qc}")

        # ---- q0: pass1 pipeline with chain atts interleaved ----
        for h in EXP_HEADS:
            att_t[(h, 0)] = psB.tile([V1, 512], f32, tag=f"a{h}", name=f"a{h}_q0")
        mb = psX.tile([128, 512], f32, tag="x1", name="mbuild")
        att2q0 = None
        sd_tiles = {}
        t2h0 = {}
        d2q = {0: dist_p(0, 0)}
        for p in range(NP):
            if p + 1 < NP:
                d2q[p + 1] = dist_p(0, p + 1)
            exps_p(0, p, d2q.pop(p))
            for h in EXP_HEADS:
                att_p(att_t[(h, 0)], h, s_store[(h, 0)], p)
            if p == 0:
                for kt in range(KT):
                    nc.tensor.matmul(mb[:, 0:3 * V1], lhsT=phi_sb[:, kt, :],
                                     rhs=pv_sb[:, kt, :],
                                     start=(kt == 0), stop=(kt == KT - 1))
                nc.vector.tensor_copy(out=m_sb[:], in_=mb[:, 0:3 * V1])
            if p % 2 == 1:
                g = p // 2
                sd_tiles[(2, 0, g)] = chain_g(2, 3, 0, g, "v")
                sd_tiles[(0, 0, g)] = chain_g(0, 1, 0, g, "v")
                if g == 0:
                    att2q0 = psX.tile([128, 512], f32, tag="x1", name="att2_q0")
                    att_t[(2, 0)] = att2q0
                att_chain_g(att2q0, 2, sd_tiles[(2, 0, g)], g)

        # evacs q0, lo heads first (their norm rows ride flat row 64)
        evac_lo(att_t[(3, 0)], 3, 0, "v")
        evac_lo(att_t[(1, 0)], 1, 0, "v")
        evac_lo(att_t[(4, 0)], 4, 0, "v")
        evac_hi(att_t[(2, 0)], 2, 0, "v")

        # att0-q0 dense (x1 after att2-q0)
        att0q0 = psX.tile([128, 512], f32, tag="x1", name="att0_q0")
        att_t[(0, 0)] = att0q0
        for g in range(NG):
            att_chain_g(att0q0, 0, sd_tiles[(0, 0, g)], g)
        evac_hi(att_t[(0, 0)], 0, 0, "v")

        # q1 dist prefetch first, then poly atts q0 claim the psB rotations
        # ahead of the q1 att tiles; q1 attends lag one pair so psB slot
        # waits cannot block the dist prefetch in the PE FIFO.
        d2q1 = {0: dist_p(1, 0), 1: dist_p(1, 1)}
        exps_p(1, 0, d2q1.pop(0))
        pt0 = patt_emit(0, ["a3", "a1", "a4"])
        evac_lo(pt0[1][0], 6, 0, "v")
        evac_hi(pt0[0][0], 5, 0, "v")
        evac_hi(pt0[2][0], 7, 0, "v")
        recip_q(0)
        for h in EXP_HEADS:
            att_t[(h, 1)] = psB.tile([V1, 512], f32, tag=f"a{h}", name=f"a{h}_q1")

        # tail-q0 rb/proj ride x1 under the q1 exp backbone
        att2q1 = None
        for p in range(1, NP):
            if p + 1 < NP:
                d2q1[p + 1] = dist_p(1, p + 1)
            exps_p(1, p, d2q1.pop(p))
            for h in EXP_HEADS:
                att_p(att_t[(h, 1)], h, s_store[(h, 1)], p - 1)
            if p % 2 == 1:
                g = p // 2
                sd_tiles[(2, 1, g)] = chain_g(2, 3, 1, g, "v")
                sd_tiles[(0, 1, g)] = chain_g(0, 1, 1, g, "v")
                if g == 1:
                    att2q1 = psX.tile([128, 512], f32, tag="x1", name="att2_q1")
                    att_t[(2, 1)] = att2q1
                    att_chain_g(att2q1, 2, sd_tiles[(2, 1, 0)], 0)
                if g >= 1:
                    att_chain_g(att2q1, 2, sd_tiles[(2, 1, g)], g)
        for h in EXP_HEADS:
            att_p(att_t[(h, 1)], h, s_store[(h, 1)], NP - 1)

        # evacs q1 for exp heads (lo roles; ACT helps at the tail)
        evac_lo(att_t[(3, 1)], 3, 1, "s")
        evac_lo(att_t[(1, 1)], 1, 1, "s")
        evac_lo(att_t[(4, 1)], 4, 1, "v")
        evac_hi(att_t[(2, 1)], 2, 1, "v")

        # ---- q1 critical chain first ----
        att0q1 = psB.tile([V1, 512], f32, tag="a1", name="att0_q1")
        att_t[(0, 1)] = att0q1
        for g in range(NG):
            att_chain_g(att0q1, 0, sd_tiles[(0, 1, g)], g)
        evac_hi(att_t[(0, 1)], 0, 1, "v")

        pt1 = patt_emit(1, ["a3", "a4", "a4"])
        evac_lo(pt1[1][0], 6, 1, "v")
        evac_hi(pt1[0][0], 5, 1, "v")
        evac_hi(pt1[2][0], 7, 1, "v")
        recip_q(1)

        rbA = psA.tile([128, PK, 512], f32, tag="d0", name="rbA_q1")
        rbB = psA.tile([128, PK, 512], f32, tag="d1", name="rbB_q1")
        rbt = [rbA[:, 0, :], rbA[:, 1, :], rbB[:, 0, :], rbB[:, 1, :]]
        for j in range(4):
            rb_j(1, j, rbt[j])
        for j in range(4):
            rbp = psX.tile([128, 512], f32, tag="x1", name=f"rb{j}_q0")
            rb_j(0, j, rbp)
            nc.vector.tensor_mul(flat[j][:, qsl(1)], flat[j][:, qsl(1)], rbt[j])
            nc.vector.tensor_mul(flat[j][:, qsl(0)], flat[j][:, qsl(0)], rbp[:])

        poA = psA.tile([128, PK, 512], f32, tag="d0", name="poA_q1")
        poB = psA.tile([128, PK, 512], f32, tag="d1", name="poB_q1")
        pot = [poA[:, 0, :], poA[:, 1, :], poB[:, 0, :], poB[:, 1, :]]
        for ot in range(4):
            proj_ot(1, ot, pot[ot], "s" if ot % 2 else "v")

        # ---- terminal q0 rb/proj last ----
        for ot in range(4):
            po = psX.tile([128, 512], f32, tag="x1", name=f"po{ot}_q0")
            proj_ot(0, ot, po, "s")

    nc.compile()
    _cache[key] = nc
    return nc


def _hilo(x, bf16):
    hi = x.astype(bf16)
    lo = (x - hi.astype(np.float32)).astype(bf16)
    return hi, lo


def _poly_feats(kp, qp, cv):
    phi = np.zeros((kp.shape[0], 128), np.float64)
    psi = np.zeros((128, qp.shape[0]), np.float64)
    kpd = kp.astype(np.float64)
    qpd = qp.astype(np.float64)
    for h, deg in POLY_DEG.items():
        c = float(cv[h])
        b0 = POLY_BASE[h]
        ek = np.exp(-c * (kpd ** 2).sum(-1))
        eq = np.exp(-c * (qpd ** 2).sum(-1))
        fact = [1.0]
        for i in range(1, deg + 1):
            fact.append(fact[-1] * i)
        for p, (i, j, kx) in enumerate(_mono_exps(deg)):
            coef = (2 * c) ** (i + j + kx) / (fact[i] * fact[j] * fact[kx])
            phi[:, b0 + p] = kpd[:, 0] ** i * kpd[:, 1] ** j * kpd[:, 2] ** kx * ek
            psi[b0 + p] = coef * qpd[:, 0] ** i * qpd[:, 1] ** j \
                * qpd[:, 2] ** kx * eq
    return phi.astype(np.float32), psi.astype(np.float32)


def _prep_core(qp, kp, vals, mask, w_out, cv, bf16, batch_cache):
    q2 = (qp * qp).sum(-1)
    one_q = np.ones(QS, np.float32)
    qa5 = np.stack([2 * qp[:, 0], 2 * qp[:, 1], 2 * qp[:, 2], -one_q, -q2])
    qa_hi, qa_lo = _hilo(qa5.astype(np.float32), bf16)
    qa15 = np.concatenate([qa_hi, qa_hi, qa_lo]).astype(np.float32)
    qa4 = np.zeros((64, QS), np.float32)
    for i in range(2):
        qa4[32 * i:32 * i + 15, :] = qa15

    if "ka4" not in batch_cache:
        k2 = (kp * kp).sum(-1)
        one_k = np.ones(LK, np.float32)
        ka5 = np.stack([kp[:, 0], kp[:, 1], kp[:, 2], k2, one_k])
        ka_hi, ka_lo = _hilo(ka5.astype(np.float32), bf16)
        ka15 = np.concatenate([ka_hi, ka_lo, ka_hi]).astype(np.float32)
        ka4 = np.zeros((64, NP, 128), np.float32)
        for kt in range(KT):
            p, i = kt // PK, kt % PK
            ka4[32 * i:32 * i + 15, p, :] = ka15[:, kt * 128:(kt + 1) * 128]
        vv = np.concatenate([vals, np.ones((LK, H, 1), np.float32)], axis=-1)
        vv = vv.copy()
        vv[mask] = 0.0
        v8 = vv.reshape(KT, 128, H, V1)
        vp_ = np.ascontiguousarray(
            v8[:, :, 0:5, :].reshape(KT, 128, 5 * V1).transpose(1, 0, 2))
        pv_ = np.ascontiguousarray(
            v8[:, :, 5:8, :].reshape(KT, 128, 3 * V1).transpose(1, 0, 2))
        batch_cache["ka4"] = ka4
        batch_cache["vp"] = vp_
        batch_cache["pv"] = pv_
    phi_f, psi_f = _poly_feats(kp, qp, cv)
    phi4 = np.ascontiguousarray(phi_f.reshape(KT, 128, 128).transpose(1, 0, 2))

    cast = lambda a: np.ascontiguousarray(a).astype(bf16)
    return {"ka4": cast(batch_cache["ka4"]), "qa4": cast(qa4),
            "vp": cast(batch_cache["vp"]), "pv": cast(batch_cache["pv"]),
            "phi": cast(phi4), "psi": cast(psi_f)}


def kernel(query_positions, key_positions, values, masked_elements,
           lengthscales, w_out, _want_trace=False):
    import ml_dtypes
    from concourse.bass_utils import run_bass_kernel_spmd

    bf16 = ml_dtypes.bfloat16
    qp = np.asarray(query_positions, np.float32)
    kp = np.asarray(key_positions, np.float32)
    vals = np.asarray(values, np.float32)
    mask = np.asarray(masked_elements).astype(bool)
    ls = np.asarray(lengthscales, np.float32)
    w = np.asarray(w_out, np.float32)

    cv = (1.0 / (ls.astype(np.float64) ** 2)).astype(np.float32)
    nc = _build(tuple(float(x) for x in cv))

    # flat row layout per j-pair: rows 0:64 <- ROWS[j][0], 64:128 <- ROWS[j][1]
    ROWS = {0: (1, 0), 1: (3, 2), 2: (4, 5), 3: (6, 7)}
    wperm = np.empty_like(w)
    sel8 = np.zeros((8, 4, 128), np.float32)
    for j in range(4):
        lo, hi = ROWS[j]
        wperm[:, j * 128:j * 128 + 64] = w[:, lo * 64:(lo + 1) * 64]
        wperm[:, j * 128 + 64:(j + 1) * 128] = w[:, hi * 64:(hi + 1) * 64]
        sel8[lo, j, :64] = 1.0
        sel8[hi, j, 64:] = 1.0
    wtp = np.ascontiguousarray(wperm.T).reshape(4, 128, OUTD).transpose(1, 0, 2)
    wtp_b = np.ascontiguousarray(wtp).astype(bf16)
    sel8_b = sel8.astype(bf16)

    in_maps = []
    caches = [{} for _ in range(B)]
    for c in range(NCORES):
        b, hf = c // 2, c % 2
        m = _prep_core(qp[b, hf * QS:(hf + 1) * QS], kp[b], vals[b], mask[b],
                       w, cv, bf16, caches[b])
        m["wt"] = wtp_b
        m["sel8"] = sel8_b
        in_maps.append(m)
    res = run_bass_kernel_spmd(nc, in_maps, core_ids=list(range(NCORES)),
                               trace=_want_trace)
    out = np.empty((B, LQ, OUTD), np.float32)
    for c in range(NCORES):
        b, hf = c // 2, c % 2
        out[b, hf * QS:(hf + 1) * QS, :] = res.results[c]["outT"].T
    if _want_trace:
        return out, res
    return out
